# revision 22
# baseline (speedup 1.0000x reference)
"""Trainium2 Bass kernel for nn_CanadarmJacob (space-arm Jacobian, bm=1 path).

Contract: kernel(**inputs) takes FULL inputs (com_list (512,256,3,7) f32,
link_pose_list (512,256,4,4,9) f32, bm scalar) and returns the FULL output
(512,256,6,7) f32. Internally shards samples across 8 NeuronCores (pure data
parallel), runs a Bass/Tile kernel per core, and gathers.

v4 design — single-engine (Vector) fp16 pipeline, (c, a, J) layout, J=128:
  - GpSimd is NOT used: it contends with the Vector engine for the SBUF port
    (measured: concurrent GpSimd streams inflate DVE ops ~4x).
  - Host pre-gathers rot = pose[:3,AXIS[a],a]*sign*0.25 and pos = pose[:3,3,:7].
  - Global 2^-8 scale folded into the M/SM/D/CD constants keeps the entire
    H_s assembly AND adjugate inside fp16 range, so the whole per-sample 3x3
    inverse chain runs at the DVE 2x fp16 rate (det accumulates in fp32).
    The scale is undone inside A = -4*adj'/det' and the -4/TM top scale.
  - Cross products via row-duplicated (x,y,z,x,y) tensors: 3 full-width ops.
  - Per-act reduction via fp16 tree adds.
  - Inputs packed into TWO dram tensors (one per HWDGE queue) to minimize
    doorbell + semaphore overhead; critical rows (pos/com/mt) lead.
  - ScalarE only does row-duplication copies, overlapped with Vector.

Math (reformulated; primes denote the 2^-8-scaled versions):
  del   = com - pos ; mdel' = M' del ; jac = rot x del
  S'_cc' = sum_a mdel'_c del_c' ; scom' = sum_a M' com ; c = scom'/(TM*2^-8)-BASE
  w'    = suffix_cumsum(mdel') ; w2' = w' - SM' c
  Hth'  = D' rot + w2' x jac ; jsmS = (-4/TM) SM jac
  H'_s  = (TM*2^-8)(cc^T - |c|^2 I) + (SS' I - S') + CD' ; A = -4 adj(H'_s)/det'
  bot   = A @ Hth' ; top = jsmS + c x bot
"""
import sys
import functools

if "/opt/trn_rl_repo" not in sys.path:
    sys.path.insert(0, "/opt/trn_rl_repo")

import numpy as np

# ---------------------------------------------------------------- constants
N_CORES = 8
P = 128          # SBUF partitions
J = 128          # samples per partition per core
N_ACT = 7
RS = 0.25        # rot pre-scale (fp16 range headroom); folded into A
SC = 2.0 ** -8   # global mass/inertia scale (fp16 range); folded into A and c

MASS = np.array([105.98, 105.98, 314.98, 279.2, 105.98, 105.98, 243.66], np.float64)
TM = float(MASS.sum() + 100000.0 + 243.66)
DIAGS = np.array([[12.19, 12.19, 3.061], [12.19, 12.19, 3.061], [15.41, 2094.71, 2103.19],
                  [9.522, 1966.28, 1966.28], [8.305, 3.061, 8.0386], [12.13, 12.13, 3.061],
                  [9.336, 44.41, 44.41]], np.float64)
D_SUF = np.cumsum(DIAGS[::-1], axis=0)[::-1]          # (7,3) suffix inertia diag
SM = np.cumsum(MASS[::-1])[::-1]                      # (7,) suffix mass
CD = DIAGS.sum(axis=0)                                # (3,)
_TF0 = np.array([[1, 0, 0, 0], [0, -1, 0, 0], [0, 0, 1.3, 6], [0, 0, 0, 1]], np.float64)
_COM0 = np.array([[1, 0, 0, 0], [0, 1, 0, 0], [0, 0, 1, 0.5], [0, 0, 0, 1]], np.float64)
BASE = (_TF0 @ _COM0)[:3, 3] * 243.66 / (100000.0 + 243.66)   # [0, 0, ~0.0162]
AXIS = np.array([2, 0, 2, 2, 2, 0, 2])
SIGN = np.array([1., 1., 1., 1., -1., 1., 1.], np.float64)

# fp16 smalls tile (smx) row indices. ccd sits right before the tree output
# (red) rows so (ccd|S_d) form a (2,3,J) group for paired ops.
CCD = 0           # 0..2: (TM*SC c)*c diag
RED = 3           # 3..11: tree output: S_d(3:6), S_o(6:9), scom(9:12)
CCO = 12          # 12..14: (TM*SC c)*roll(c) off-diag (xy,yz,zx)
CSQ_R, SS_R = 15, 16   # adjacent pair (csq, SS)
UVR = 17          # 17..22: u(17:20) = ccd-csq, v'(20:23) = S_d-SS
HS0 = 23          # 23..28: h0,h1,h2 (diag), h3,h4,h5 (xy,yz,zx)
M1R = 29          # 29..30 scratch pair
M2R = 31          # 31..32 scratch pair
NSF = 33
# fp32 det tile rows
D1R, D2R, D3R, DET_R, RDN_R = 0, 1, 2, 3, 4


def _emit(nc, tc, ctx, dram):
    from concourse import mybir

    f16 = mybir.dt.float16
    f32 = mybir.dt.float32
    OP = mybir.AluOpType
    V = nc.vector
    S = nc.scalar

    pool = ctx.enter_context(tc.tile_pool(name="main", bufs=1))

    def T(name, shape, dtype=f16, **kw):
        return pool.tile([P] + shape, dtype, name=name, **kw)

    # packed inputs: crp rows = com(0:3) rot(3:6) rotdup(6:8) cdt(8, cols 0:3)
    #                pmd rows = pos(0:3) mt(3) smt(4) smtS(5) dt(6:9)
    crp = T("crp", [9, 7, J])
    pmd = T("pmd", [9, 7, J])
    del5 = T("del5", [5, 7, J])
    mdelw = T("mdelw", [3, 7, J])   # mdel' -> suffix-cumsum -> w' (in place)
    prod = T("prod", [9, 7, J])     # Sd'(3), So'(3), mcom'(3)
    t1 = T("t1", [9, 3, J])
    t2 = T("t2", [9, J])
    t3 = T("t3", [9, J])
    cb5 = T("cb5", [5, J])          # c rows (x,y,z,x,y)
    cT = T("cT", [3, J])            # (TM*SC)*c
    smx = T("smx", [NSF, J])
    red = smx[:, RED:RED + 9]       # tree output lives inside smx
    adj9 = T("adj9", [9, J])        # (c,r): A11,A12,A13,A12,A22,A23,A13,A23,A33
    detf = T("detf", [5, J], f32)
    A9 = T("A9", [3, 3, J])         # -4 * adj' / det'
    SMc = T("SMc", [3, 7, J])
    w25 = T("w25", [5, 7, J])
    jacA = T("jacA", [3, 7, J])
    jacB = T("jacB", [3, 7, J])
    jac5 = T("jac5", [5, 7, J])
    Ht = T("Ht", [3, 7, J])
    jsmS = T("jsmS", [3, 7, J])
    botp2 = T("botp2", [3, 7, J])
    bot5 = T("bot5", [5, 7, J])
    tu = T("tu", [3, 7, J])
    tu2 = T("tu2", [3, 7, J])
    top = T("top", [3, 7, J])
    # aliases (same tile object => dependency tracking stays correct)
    HtA = jacA                # dead after jac5 sub
    HtB = jacB
    Drot = del5               # rows 0:3 reused after the last del5 read (jacB)
    botP = prod               # (9,7,J) viewed r-major; dead after red

    com = crp[:, 0:3]
    rot5 = crp[:, 3:8]
    cdt = crp[:, 8, 0:3]            # (P,3,J)
    pos = pmd[:, 0:3]
    dt = pmd[:, 6:9]

    # ---------------- DMA in (split per HWDGE queue) ----------------
    # First compute waits only on the critical slices (pos/mt/com); those are
    # load-balanced across the two queues (the scalar queue starts ~2.7us
    # later, so it gets fewer critical bytes).
    nc.sync.dma_start(out=pmd[:, 0:4], in_=dram["pmd"][:, 0:4])         # pos,mt
    nc.gpsimd.dma_start(out=crp[:, 0:3], in_=dram["crp"][:, 0:3])       # com (SWDGE)
    nc.scalar.dma_start(out=crp[:, 3:9], in_=dram["crp"][:, 3:9])       # rot,cdt
    nc.sync.dma_start(out=pmd[:, 4:9], in_=dram["pmd"][:, 4:9])         # rest

    def bc(ap, shape):
        return ap.broadcast_to(shape)

    mtb = bc(pmd[:, 3].unsqueeze(1), (P, 3, 7, J))
    smtb = bc(pmd[:, 4].unsqueeze(1), (P, 3, 7, J))
    smtSb = bc(pmd[:, 5].unsqueeze(1), (P, 3, 7, J))

    # ---------------- early fp16 stages ----------------
    V.tensor_tensor(out=del5[:, 0:3], in0=com, in1=pos, op=OP.subtract)
    S.copy(out=del5[:, 3:5], in_=del5[:, 0:2])
    V.tensor_tensor(out=mdelw[:], in0=mtb, in1=del5[:, 0:3], op=OP.mult)
    V.tensor_tensor(out=prod[:, 0:3], in0=mdelw[:], in1=del5[:, 0:3], op=OP.mult)
    V.tensor_tensor(out=prod[:, 3:6], in0=mdelw[:], in1=del5[:, 1:4], op=OP.mult)
    V.tensor_tensor(out=prod[:, 6:9], in0=mtb, in1=com, op=OP.mult)
    V.tensor_tensor(out=jacA[:], in0=rot5[:, 1:4], in1=del5[:, 2:5], op=OP.mult)
    V.tensor_tensor(out=jacB[:], in0=rot5[:, 2:5], in1=del5[:, 1:4], op=OP.mult)
    V.tensor_tensor(out=jac5[:, 0:3], in0=jacA[:], in1=jacB[:], op=OP.subtract)
    S.copy(out=jac5[:, 3:5], in_=jac5[:, 0:2])

    # tree reduction over acts: 7 = (0:3 + 3:6), pairwise, + col 6
    V.tensor_tensor(out=t1[:], in0=prod[:, :, 0:3], in1=prod[:, :, 3:6], op=OP.add)
    V.tensor_tensor(out=t2[:], in0=t1[:, :, 0], in1=t1[:, :, 1], op=OP.add)
    V.tensor_tensor(out=t3[:], in0=t2[:], in1=t1[:, :, 2], op=OP.add)
    V.tensor_tensor(out=red, in0=t3[:], in1=prod[:, :, 6], op=OP.add)

    # c (fp16) = red[6:9]/(TM*SC) - BASE ; cT = (TM*SC)*c
    V.tensor_scalar(out=cb5[:, 0:2], in0=red[:, 6:8], scalar1=1.0 / (TM * SC),
                    scalar2=None, op0=OP.mult)
    V.tensor_scalar(out=cb5[:, 2], in0=red[:, 8], scalar1=1.0 / (TM * SC),
                    scalar2=float(BASE[2]), op0=OP.mult, op1=OP.subtract)
    S.copy(out=cb5[:, 3:5], in_=cb5[:, 0:2])
    V.tensor_scalar(out=cT[:], in0=cb5[:, 0:3], scalar1=TM * SC, scalar2=None,
                    op0=OP.mult)

    # suffix cumsum over acts: mdelw becomes w'
    for k in range(5, -1, -1):
        V.tensor_tensor(out=mdelw[:, :, k], in0=mdelw[:, :, k],
                        in1=mdelw[:, :, k + 1], op=OP.add)

    # w2' = w' - SM' (x) c
    cb3a = bc(cb5[:, 0:3].unsqueeze(2), (P, 3, 7, J))
    V.tensor_tensor(out=SMc[:], in0=smtb, in1=cb3a, op=OP.mult)
    V.tensor_tensor(out=w25[:, 0:3], in0=mdelw[:], in1=SMc[:], op=OP.subtract)
    S.copy(out=w25[:, 3:5], in_=w25[:, 0:2])

    # ---------------- H'_s rows (fp16) ----------------
    # (ccd | S_d) rows 0:6 form a (2,3,J) group: one op computes (csq,SS)
    # partials and one computes (u, v') = (ccd,S_d) - (csq,SS)_bc.
    V.tensor_tensor(out=smx[:, CCD:CCD + 3], in0=cT[:], in1=cb5[:, 0:3], op=OP.mult)
    V.tensor_tensor(out=smx[:, CCO:CCO + 3], in0=cT[:], in1=cb5[:, 1:4], op=OP.mult)
    gv = smx[:, 0:6].rearrange("p (g r) j -> p g r j", g=2)
    x2 = smx[:, M1R:M1R + 2]
    V.tensor_tensor(out=x2, in0=gv[:, :, 0], in1=gv[:, :, 1], op=OP.add)
    V.tensor_tensor(out=smx[:, CSQ_R:CSQ_R + 2], in0=x2, in1=gv[:, :, 2], op=OP.add)
    csqss_b = bc(smx[:, CSQ_R:CSQ_R + 2].unsqueeze(2), (P, 2, 3, J))
    uvv = smx[:, UVR:UVR + 6].rearrange("p (g r) j -> p g r j", g=2)
    V.tensor_tensor(out=uvv, in0=gv, in1=csqss_b, op=OP.subtract)
    s1 = smx[:, M1R:M1R + 3]        # u - v' (3,J) scratch (overwrites x2 rows)
    V.tensor_tensor(out=s1, in0=smx[:, UVR:UVR + 3], in1=smx[:, UVR + 3:UVR + 6],
                    op=OP.subtract)
    V.tensor_tensor(out=smx[:, HS0:HS0 + 3], in0=s1, in1=cdt, op=OP.add)
    V.tensor_tensor(out=smx[:, HS0 + 3:HS0 + 6], in0=smx[:, CCO:CCO + 3],
                    in1=red[:, 3:6], op=OP.subtract)

    # ---------------- Htheta (fp16) ----------------
    V.tensor_tensor(out=HtA[:], in0=w25[:, 1:4], in1=jac5[:, 2:5], op=OP.mult)
    V.tensor_tensor(out=HtB[:], in0=w25[:, 2:5], in1=jac5[:, 1:4], op=OP.mult)
    V.tensor_tensor(out=Ht[:], in0=HtA[:], in1=HtB[:], op=OP.subtract)
    V.tensor_tensor(out=Drot[:, 0:3], in0=dt, in1=rot5[:, 0:3], op=OP.mult)
    V.tensor_tensor(out=Ht[:], in0=Ht[:], in1=Drot[:, 0:3], op=OP.add)
    V.tensor_tensor(out=jsmS[:], in0=smtSb, in1=jac5[:, 0:3], op=OP.mult)

    # ---------------- adjugate (fp16) + det (fp32 accum) ----------------
    h = lambda i: smx[:, HS0 + i]
    b2 = lambda ap: bc(ap.unsqueeze(1), (P, 2, J))
    m1 = smx[:, M1R:M1R + 2]
    m2 = smx[:, M2R:M2R + 2]
    # A11 = h1 h2 - h4^2 ; A22 = h0 h2 - h5^2   -> adj rows 0, 4
    V.tensor_tensor(out=m1, in0=smx[:, HS0 + 1:HS0 - 1:-1], in1=b2(h(2)), op=OP.mult)
    V.tensor_tensor(out=m2, in0=smx[:, HS0 + 4:HS0 + 6], in1=smx[:, HS0 + 4:HS0 + 6],
                    op=OP.mult)
    V.tensor_tensor(out=adj9[:, 0:5:4], in0=m1, in1=m2, op=OP.subtract)
    # A13 = h3 h4 - h1 h5 ; A23 = h3 h5 - h0 h4 -> adj rows 2, 5
    V.tensor_tensor(out=m1, in0=b2(h(3)), in1=smx[:, HS0 + 4:HS0 + 6], op=OP.mult)
    V.tensor_tensor(out=m2, in0=smx[:, HS0 + 1:HS0 - 1:-1],
                    in1=smx[:, HS0 + 5:HS0 + 3:-1], op=OP.mult)
    V.tensor_tensor(out=adj9[:, 2:6:3], in0=m1, in1=m2, op=OP.subtract)
    # A12 = h5 h4 - h3 h2 -> row 1 ; A33 = h0 h1 - h3^2 -> row 8
    V.tensor_tensor(out=m1, in0=smx[:, HS0 + 5:HS0 - 1:-5], in1=smx[:, HS0 + 4:HS0:-3],
                    op=OP.mult)
    V.tensor_tensor(out=m2, in0=b2(h(3)), in1=smx[:, HS0 + 2:HS0 + 4], op=OP.mult)
    V.tensor_tensor(out=adj9[:, 1:9:7], in0=m1, in1=m2, op=OP.subtract)
    # det' = h0 A11 + h3 A12 + h5 A13 (fp32 accumulation)
    V.tensor_tensor(out=detf[:, D1R], in0=h(0), in1=adj9[:, 0], op=OP.mult)
    V.tensor_tensor(out=detf[:, D2R], in0=h(3), in1=adj9[:, 1], op=OP.mult)
    V.tensor_tensor(out=detf[:, D3R], in0=h(5), in1=adj9[:, 2], op=OP.mult)
    V.tensor_tensor(out=detf[:, DET_R], in0=detf[:, D1R], in1=detf[:, D2R], op=OP.add)
    V.tensor_tensor(out=detf[:, DET_R], in0=detf[:, DET_R], in1=detf[:, D3R],
                    op=OP.add)
    # mirror rows (ScalarE): A12->3, A13->6, A23->7
    S.copy(out=adj9[:, 3], in_=adj9[:, 1])
    S.copy(out=adj9[:, 6], in_=adj9[:, 2])
    S.copy(out=adj9[:, 7], in_=adj9[:, 5])
    V.reciprocal_approx_fast(out=detf[:, RDN_R], in_=detf[:, DET_R])
    rdn9 = bc(detf[:, RDN_R].unsqueeze(1), (P, 9, J))
    V.scalar_tensor_tensor(out=A9[:].rearrange("p a b j -> p (a b) j"),
                           in0=adj9[:], scalar=-4.0, in1=rdn9,
                           op0=OP.mult, op1=OP.mult)

    # ---------------- bot = A @ Ht' ----------------
    botP4 = botP[:].rearrange("p (r c) a j -> p r c a j", r=3)
    for r in range(3):
        V.tensor_tensor(out=botP4[:, r],
                        in0=bc(A9[:, :, r].unsqueeze(2), (P, 3, 7, J)),
                        in1=bc(Ht[:, r].unsqueeze(1), (P, 3, 7, J)), op=OP.mult)
    V.tensor_tensor(out=botp2[:], in0=botP4[:, 0], in1=botP4[:, 1], op=OP.add)
    V.tensor_tensor(out=bot5[:, 0:3], in0=botp2[:], in1=botP4[:, 2], op=OP.add)
    V.tensor_copy(out=bot5[:, 3:5], in_=bot5[:, 0:2])
    nc.scalar.dma_start(out=dram["outB"][:], in_=bot5[:, 0:3])

    # ---------------- top = jsmS + c x bot ----------------
    cb14 = bc(cb5[:, 1:4].unsqueeze(2), (P, 3, 7, J))
    cb25 = bc(cb5[:, 2:5].unsqueeze(2), (P, 3, 7, J))
    V.tensor_tensor(out=tu[:], in0=cb14, in1=bot5[:, 2:5], op=OP.mult)
    V.tensor_tensor(out=tu2[:], in0=cb25, in1=bot5[:, 1:4], op=OP.mult)
    V.tensor_tensor(out=tu[:], in0=tu[:], in1=tu2[:], op=OP.subtract)
    # final add split on acts so the first outT half DMAs during the second add
    V.tensor_tensor(out=top[:, :, 0:4], in0=jsmS[:, :, 0:4], in1=tu[:, :, 0:4],
                    op=OP.add)
    nc.sync.dma_start(out=dram["outT"][:, :, 0:4], in_=top[:, :, 0:4])
    V.tensor_tensor(out=top[:, :, 4:7], in0=jsmS[:, :, 4:7], in1=tu[:, :, 4:7],
                    op=OP.add)
    nc.scalar.dma_start(out=dram["outT"][:, :, 4:7], in_=top[:, :, 4:7])


@functools.lru_cache(maxsize=1)
def _program():
    from contextlib import ExitStack
    import concourse.bacc as bacc
    import concourse.tile as tile
    from concourse import mybir

    f16 = mybir.dt.float16
    nc = bacc.Bacc("TRN2", target_bir_lowering=False, debug=False)
    dram = {
        "crp": nc.dram_tensor("crp", [P, 9, 7, J], f16, kind="ExternalInput"),
        "pmd": nc.dram_tensor("pmd", [P, 9, 7, J], f16, kind="ExternalInput"),
        "outB": nc.dram_tensor("outB", [P, 3, 7, J], f16, kind="ExternalOutput"),
        "outT": nc.dram_tensor("outT", [P, 3, 7, J], f16, kind="ExternalOutput"),
    }
    with tile.TileContext(nc) as tc:
        with ExitStack() as ctx:
            _emit(nc, tc, ctx, dram)
    nc.compile()
    return nc


@functools.lru_cache(maxsize=1)
def _pmd_consts():
    """pmd const rows 3..8 (P,6,7,J) and the crp cdt row (P,7,J), fp16."""
    rows = np.zeros((6, 7, J), np.float32)
    rows[0] = (MASS * SC)[:, None]
    rows[1] = (SM * SC)[:, None]
    rows[2] = (SM * (-4.0 / TM))[:, None]
    rows[3:6] = (D_SUF.T * SC)[:, :, None]
    cdt_row = np.zeros((7, J), np.float32)
    cdt_row[0:3] = (CD * SC)[:, None]
    pmd_c = np.broadcast_to(rows.astype(np.float16)[None], (P, 6, 7, J))
    cdt_c = np.broadcast_to(cdt_row.astype(np.float16)[None], (P, 7, J))
    return pmd_c, cdt_c


def prepare_in_maps(com_list, pose_list):
    """Host-side marshalling: gather/scale/cast/transpose/pack."""
    com_r = com_list.reshape(N_CORES, P, J, 3, N_ACT)
    com16 = com_r.transpose(0, 1, 3, 4, 2).astype(np.float16)
    pose_r = pose_list.reshape(N_CORES, P, J, 4, 4, 9)
    rotg = pose_r[:, :, :, :3, AXIS, np.arange(N_ACT)]       # (k,P,J,3,7)
    rot16 = (rotg * (SIGN * RS).astype(np.float32)).transpose(0, 1, 3, 4, 2) \
        .astype(np.float16)
    posg = pose_r[:, :, :, :3, 3, :N_ACT]
    pos16 = posg.transpose(0, 1, 3, 4, 2).astype(np.float16)
    pmd_c, cdt_c = _pmd_consts()

    crp = np.empty((N_CORES, P, 9, 7, J), np.float16)
    pmd = np.empty((N_CORES, P, 9, 7, J), np.float16)
    crp[:, :, 0:3] = com16
    crp[:, :, 3:6] = rot16
    crp[:, :, 6:8] = rot16[:, :, 0:2]
    crp[:, :, 8] = cdt_c[None]
    pmd[:, :, 0:3] = pos16
    pmd[:, :, 3:9] = pmd_c[None]
    return [dict(crp=crp[k], pmd=pmd[k]) for k in range(N_CORES)]


def postprocess(results):
    """(k, P, 3, 7, J) fp16 outT/outB -> (512,256,6,7) fp32."""
    out = np.empty((N_CORES, P, J, 6, N_ACT), np.float32)
    for k in range(N_CORES):
        out[k, :, :, 0:3] = results[k]["outT"].astype(np.float32).transpose(0, 3, 1, 2)
        out[k, :, :, 3:6] = results[k]["outB"].astype(np.float32).transpose(0, 3, 1, 2)
    return out.reshape(512, 256, 6, N_ACT)


def _kernel_bm0(com, pose):
    # bm=0 path (not exercised by the shipped setup_inputs; numpy fallback)
    rot = pose[:, :, :3, 2, :N_ACT].copy()
    rot[..., 1] = pose[:, :, :3, 0, 1]
    rot[..., 5] = pose[:, :, :3, 0, 5]
    rot[..., 4] *= -1.0
    delp = pose[:, :, :3, 3, -2][..., None] - pose[:, :, :3, 3, :N_ACT]
    jt = np.cross(rot, delp, axis=2)
    return np.concatenate([jt, rot], axis=2).astype(np.float32)


def kernel(com_list, link_pose_list, bm):
    com_list = np.ascontiguousarray(com_list, dtype=np.float32)
    link_pose_list = np.ascontiguousarray(link_pose_list, dtype=np.float32)
    if not int(bm):
        return _kernel_bm0(com_list, link_pose_list)

    from concourse.bass_utils import run_bass_kernel_spmd

    nc = _program()
    in_maps = prepare_in_maps(com_list, link_pose_list)
    res = run_bass_kernel_spmd(nc, in_maps, core_ids=list(range(N_CORES)))
    return postprocess(res.results)


# revision 27
# speedup vs baseline: 1.0756x; 1.0756x over previous
"""Trainium2 Bass kernel for nn_CanadarmJacob (space-arm Jacobian, bm=1 path).

Contract: kernel(**inputs) takes FULL inputs (com_list (512,256,3,7) f32,
link_pose_list (512,256,4,4,9) f32, bm scalar) and returns the FULL output
(512,256,6,7) f32. Internally shards samples across 8 NeuronCores (pure data
parallel), runs a Bass/Tile kernel per core, and gathers.

v4 design — single-engine (Vector) fp16 pipeline, (c, a, J) layout, J=128:
  - GpSimd is NOT used: it contends with the Vector engine for the SBUF port
    (measured: concurrent GpSimd streams inflate DVE ops ~4x).
  - Host pre-gathers rot = pose[:3,AXIS[a],a]*sign*0.25 and pos = pose[:3,3,:7].
  - Global 2^-8 scale folded into the M/SM/D/CD constants keeps the entire
    H_s assembly AND adjugate inside fp16 range, so the whole per-sample 3x3
    inverse chain runs at the DVE 2x fp16 rate (det accumulates in fp32).
    The scale is undone inside A = -4*adj'/det' and the -4/TM top scale.
  - Cross products via row-duplicated (x,y,z,x,y) tensors: 3 full-width ops.
  - Per-act reduction via fp16 tree adds.
  - Inputs packed into TWO dram tensors (one per HWDGE queue) to minimize
    doorbell + semaphore overhead; critical rows (pos/com/mt) lead.
  - ScalarE only does row-duplication copies, overlapped with Vector.

Math (reformulated; primes denote the 2^-8-scaled versions):
  del   = com - pos ; mdel' = M' del ; jac = rot x del
  S'_cc' = sum_a mdel'_c del_c' ; scom' = sum_a M' com ; c = scom'/(TM*2^-8)-BASE
  w'    = suffix_cumsum(mdel') ; w2' = w' - SM' c
  Hth'  = D' rot + w2' x jac ; jsmS = (-4/TM) SM jac
  H'_s  = (TM*2^-8)(cc^T - |c|^2 I) + (SS' I - S') + CD' ; A = -4 adj(H'_s)/det'
  bot   = A @ Hth' ; top = jsmS + c x bot
"""
import sys
import functools

if "/opt/trn_rl_repo" not in sys.path:
    sys.path.insert(0, "/opt/trn_rl_repo")

import numpy as np

# ---------------------------------------------------------------- constants
N_CORES = 8
P = 128          # SBUF partitions
J = 128          # samples per partition per core
N_ACT = 7
RS = 0.25        # rot pre-scale (fp16 range headroom); folded into A
SC = 2.0 ** -8   # global mass/inertia scale (fp16 range); folded into A and c

MASS = np.array([105.98, 105.98, 314.98, 279.2, 105.98, 105.98, 243.66], np.float64)
TM = float(MASS.sum() + 100000.0 + 243.66)
DIAGS = np.array([[12.19, 12.19, 3.061], [12.19, 12.19, 3.061], [15.41, 2094.71, 2103.19],
                  [9.522, 1966.28, 1966.28], [8.305, 3.061, 8.0386], [12.13, 12.13, 3.061],
                  [9.336, 44.41, 44.41]], np.float64)
D_SUF = np.cumsum(DIAGS[::-1], axis=0)[::-1]          # (7,3) suffix inertia diag
SM = np.cumsum(MASS[::-1])[::-1]                      # (7,) suffix mass
CD = DIAGS.sum(axis=0)                                # (3,)
_TF0 = np.array([[1, 0, 0, 0], [0, -1, 0, 0], [0, 0, 1.3, 6], [0, 0, 0, 1]], np.float64)
_COM0 = np.array([[1, 0, 0, 0], [0, 1, 0, 0], [0, 0, 1, 0.5], [0, 0, 0, 1]], np.float64)
BASE = (_TF0 @ _COM0)[:3, 3] * 243.66 / (100000.0 + 243.66)   # [0, 0, ~0.0162]
AXIS = np.array([2, 0, 2, 2, 2, 0, 2])
SIGN = np.array([1., 1., 1., 1., -1., 1., 1.], np.float64)

# fp16 smalls tile (smx) row indices. ccd sits right before the tree output
# (red) rows so (ccd|S_d) form a (2,3,J) group for paired ops.
CCD = 0           # 0..2: (TM*SC c)*c diag
RED = 3           # 3..11: tree output: S_d(3:6), S_o(6:9), scom(9:12)
CCO = 12          # 12..14: (TM*SC c)*roll(c) off-diag (xy,yz,zx)
CSQ_R, SS_R = 15, 16   # adjacent pair (csq, SS)
UVR = 17          # 17..22: u(17:20) = ccd-csq, v'(20:23) = S_d-SS
HS0 = 23          # 23..28: h0,h1,h2 (diag), h3,h4,h5 (xy,yz,zx)
M1R = 29          # 29..30 scratch pair
M2R = 31          # 31..32 scratch pair
NSF = 33
# fp32 det tile rows
D1R, D2R, D3R, DET_R, RDN_R = 0, 1, 2, 3, 4


def _emit(nc, tc, ctx, dram):
    from concourse import mybir

    f16 = mybir.dt.float16
    f32 = mybir.dt.float32
    OP = mybir.AluOpType
    V = nc.vector
    S = nc.scalar

    pool = ctx.enter_context(tc.tile_pool(name="main", bufs=1))

    def T(name, shape, dtype=f16, **kw):
        return pool.tile([P] + shape, dtype, name=name, **kw)

    # packed inputs: crp rows = com(0:3) rot(3:6) rotdup(6:8) cdt(8, cols 0:3)
    #                pmd rows = pos(0:3) mt(3) smt(4) smtS(5) dt(6:9)
    crp = T("crp", [9, 7, J])
    pmd = T("pmd", [9, 7, J])
    del5 = T("del5", [5, 7, J])
    mdelw = T("mdelw", [3, 7, J])   # mdel' -> suffix-cumsum -> w' (in place)
    prod = T("prod", [9, 7, J])     # Sd'(3), So'(3), mcom'(3)
    t1 = T("t1", [9, 3, J])
    t2 = T("t2", [9, J])
    t3 = T("t3", [9, J])
    cb5 = T("cb5", [5, J])          # c rows (x,y,z,x,y)
    cT = T("cT", [3, J])            # (TM*SC)*c
    smx = T("smx", [NSF, J])
    red = smx[:, RED:RED + 9]       # tree output lives inside smx
    adj9 = T("adj9", [9, J])        # (c,r): A11,A12,A13,A12,A22,A23,A13,A23,A33
    detf = T("detf", [5, J], f32)
    A9 = T("A9", [3, 3, J])         # -4 * adj' / det'
    SMc = T("SMc", [3, 7, J])
    w25 = T("w25", [5, 7, J])
    jacAB = T("jacAB", [2, 3, 7, J])
    jac5 = T("jac5", [5, 7, J])
    Ht = T("Ht", [3, 7, J])
    jsmS = T("jsmS", [3, 7, J])
    botp2 = T("botp2", [3, 7, J])
    bot5 = T("bot5", [5, 7, J])
    tu = T("tu", [3, 7, J])
    tu2 = T("tu2", [3, 7, J])
    top = T("top", [3, 7, J])
    # aliases (same tile object => dependency tracking stays correct)
    Drot = del5               # rows 0:3 reused after the last del5 read (jacB)
    botP = prod               # (9,7,J) viewed r-major; dead after red

    def wap(base, dims):
        """Overlapping-window AP: keep base's tile/offset, override dims."""
        import bass_rust
        c = base.copy()
        c.ap = bass_rust.VecI64Pair(dims)
        return c

    R = 7 * J                 # one (7,J) row group, elements
    PS5 = 5 * R               # partition stride of a (5,7,J) tile

    com = crp[:, 0:3]
    rot5 = crp[:, 3:8]
    cdt = crp[:, 8, 0:3]            # (P,3,J)
    pos = pmd[:, 0:3]
    dt = pmd[:, 6:9]

    # ---------------- DMA in (split per HWDGE queue) ----------------
    # First compute waits only on the critical slices (pos/mt/com); those are
    # load-balanced across the two queues (the scalar queue starts ~2.7us
    # later, so it gets fewer critical bytes).
    nc.sync.dma_start(out=pmd[:, 0:4], in_=dram["pmd"][:, 0:4])         # pos,mt
    nc.sync.dma_start(out=crp[0:32, 0:3], in_=dram["crp"][0:32, 0:3])   # com lo
    nc.scalar.dma_start(out=crp[32:128, 0:3], in_=dram["crp"][32:128, 0:3])
    nc.scalar.dma_start(out=crp[:, 3:9], in_=dram["crp"][:, 3:9])       # rot,cdt
    nc.sync.dma_start(out=pmd[:, 4:9], in_=dram["pmd"][:, 4:9])         # rest

    def bc(ap, shape):
        return ap.broadcast_to(shape)

    mtb = bc(pmd[:, 3].unsqueeze(1), (P, 3, 7, J))
    smtb = bc(pmd[:, 4].unsqueeze(1), (P, 3, 7, J))
    smtSb = bc(pmd[:, 5].unsqueeze(1), (P, 3, 7, J))

    # ---------------- early fp16 stages ----------------
    V.tensor_tensor(out=del5[:, 0:3], in0=com, in1=pos, op=OP.subtract)
    S.copy(out=del5[:, 3:5], in_=del5[:, 0:2])
    V.tensor_tensor(out=mdelw[:], in0=mtb, in1=del5[:, 0:3], op=OP.mult)
    # Sd & So products in one instruction via an overlapping del5 window AP:
    # group g reads del5 rows (g, g+1, g+2) flattened (g=0 -> Sd, g=1 -> So).
    mdel_f = mdelw[:].rearrange("p c a j -> p (c a j)")
    V.tensor_tensor(out=prod[:, 0:6].rearrange("p (g c) a j -> p g (c a j)", g=2),
                    in0=bc(mdel_f.unsqueeze(1), (P, 2, 3 * R)),
                    in1=wap(del5[:, 0:3], [[PS5, P], [R, 2], [1, 3 * R]]),
                    op=OP.mult)
    V.tensor_tensor(out=prod[:, 6:9], in0=mtb, in1=com, op=OP.mult)
    # jacA/jacB in one instruction: in0 windows rot rows (1:4 | 2:5),
    # in1 windows del rows (2:5 | 1:4) (negative group stride).
    jacAB_f = jacAB[:].rearrange("p g c a j -> p g (c a j)")
    V.tensor_tensor(out=jacAB_f,
                    in0=wap(rot5[:, 1:4], [[9 * R, P], [R, 2], [1, 3 * R]]),
                    in1=wap(del5[:, 2:5], [[PS5, P], [-R, 2], [1, 3 * R]]),
                    op=OP.mult)
    V.tensor_tensor(out=jac5[:, 0:3], in0=jacAB[:, 0], in1=jacAB[:, 1],
                    op=OP.subtract)
    S.copy(out=jac5[:, 3:5], in_=jac5[:, 0:2])

    # tree reduction over acts: 7 = (0:3 + 3:6), pairwise, + col 6
    V.tensor_tensor(out=t1[:], in0=prod[:, :, 0:3], in1=prod[:, :, 3:6], op=OP.add)
    V.tensor_tensor(out=t2[:], in0=t1[:, :, 0], in1=t1[:, :, 1], op=OP.add)
    V.tensor_tensor(out=t3[:], in0=t2[:], in1=t1[:, :, 2], op=OP.add)
    V.tensor_tensor(out=red, in0=t3[:], in1=prod[:, :, 6], op=OP.add)

    # c (fp16) = red[6:9]/(TM*SC) - BASE ; cT = (TM*SC)*c
    V.tensor_scalar(out=cb5[:, 0:2], in0=red[:, 6:8], scalar1=1.0 / (TM * SC),
                    scalar2=None, op0=OP.mult)
    V.tensor_scalar(out=cb5[:, 2], in0=red[:, 8], scalar1=1.0 / (TM * SC),
                    scalar2=float(BASE[2]), op0=OP.mult, op1=OP.subtract)
    S.copy(out=cb5[:, 3:5], in_=cb5[:, 0:2])
    V.tensor_scalar(out=cT[:], in0=cb5[:, 0:3], scalar1=TM * SC, scalar2=None,
                    op0=OP.mult)

    # suffix cumsum over acts: mdelw becomes w'
    for k in range(5, -1, -1):
        V.tensor_tensor(out=mdelw[:, :, k], in0=mdelw[:, :, k],
                        in1=mdelw[:, :, k + 1], op=OP.add)

    # w2' = w' - SM' (x) c
    cb3a = bc(cb5[:, 0:3].unsqueeze(2), (P, 3, 7, J))
    V.tensor_tensor(out=SMc[:], in0=smtb, in1=cb3a, op=OP.mult)
    V.tensor_tensor(out=w25[:, 0:3], in0=mdelw[:], in1=SMc[:], op=OP.subtract)
    S.copy(out=w25[:, 3:5], in_=w25[:, 0:2])

    # ---------------- H'_s rows (fp16) ----------------
    # (ccd | S_d) rows 0:6 form a (2,3,J) group: one op computes (csq,SS)
    # partials and one computes (u, v') = (ccd,S_d) - (csq,SS)_bc.
    V.tensor_tensor(out=smx[:, CCD:CCD + 3], in0=cT[:], in1=cb5[:, 0:3], op=OP.mult)
    V.tensor_tensor(out=smx[:, CCO:CCO + 3], in0=cT[:], in1=cb5[:, 1:4], op=OP.mult)
    gv = smx[:, 0:6].rearrange("p (g r) j -> p g r j", g=2)
    x2 = smx[:, M1R:M1R + 2]
    V.tensor_tensor(out=x2, in0=gv[:, :, 0], in1=gv[:, :, 1], op=OP.add)
    V.tensor_tensor(out=smx[:, CSQ_R:CSQ_R + 2], in0=x2, in1=gv[:, :, 2], op=OP.add)
    csqss_b = bc(smx[:, CSQ_R:CSQ_R + 2].unsqueeze(2), (P, 2, 3, J))
    uvv = smx[:, UVR:UVR + 6].rearrange("p (g r) j -> p g r j", g=2)
    V.tensor_tensor(out=uvv, in0=gv, in1=csqss_b, op=OP.subtract)
    s1 = smx[:, M1R:M1R + 3]        # u - v' (3,J) scratch (overwrites x2 rows)
    V.tensor_tensor(out=s1, in0=smx[:, UVR:UVR + 3], in1=smx[:, UVR + 3:UVR + 6],
                    op=OP.subtract)
    V.tensor_tensor(out=smx[:, HS0:HS0 + 3], in0=s1, in1=cdt, op=OP.add)
    V.tensor_tensor(out=smx[:, HS0 + 3:HS0 + 6], in0=smx[:, CCO:CCO + 3],
                    in1=red[:, 3:6], op=OP.subtract)

    # ---------------- Htheta (fp16) ----------------
    # HtA/HtB in one instruction (windows over w25 / jac5, reusing jacAB)
    V.tensor_tensor(out=jacAB_f,
                    in0=wap(w25[:, 1:4], [[PS5, P], [R, 2], [1, 3 * R]]),
                    in1=wap(jac5[:, 2:5], [[PS5, P], [-R, 2], [1, 3 * R]]),
                    op=OP.mult)
    V.tensor_tensor(out=Ht[:], in0=jacAB[:, 0], in1=jacAB[:, 1], op=OP.subtract)
    V.tensor_tensor(out=Drot[:, 0:3], in0=dt, in1=rot5[:, 0:3], op=OP.mult)
    V.tensor_tensor(out=Ht[:], in0=Ht[:], in1=Drot[:, 0:3], op=OP.add)
    V.tensor_tensor(out=jsmS[:], in0=smtSb, in1=jac5[:, 0:3], op=OP.mult)

    # ---------------- adjugate (fp16) + det (fp32 accum) ----------------
    h = lambda i: smx[:, HS0 + i]
    b2 = lambda ap: bc(ap.unsqueeze(1), (P, 2, J))
    m1 = smx[:, M1R:M1R + 2]
    m2 = smx[:, M2R:M2R + 2]
    # A11 = h1 h2 - h4^2 ; A22 = h0 h2 - h5^2   -> adj rows 0, 4
    V.tensor_tensor(out=m1, in0=smx[:, HS0 + 1:HS0 - 1:-1], in1=b2(h(2)), op=OP.mult)
    V.tensor_tensor(out=m2, in0=smx[:, HS0 + 4:HS0 + 6], in1=smx[:, HS0 + 4:HS0 + 6],
                    op=OP.mult)
    V.tensor_tensor(out=adj9[:, 0:5:4], in0=m1, in1=m2, op=OP.subtract)
    # A13 = h3 h4 - h1 h5 ; A23 = h3 h5 - h0 h4 -> adj rows 2, 5
    V.tensor_tensor(out=m1, in0=b2(h(3)), in1=smx[:, HS0 + 4:HS0 + 6], op=OP.mult)
    V.tensor_tensor(out=m2, in0=smx[:, HS0 + 1:HS0 - 1:-1],
                    in1=smx[:, HS0 + 5:HS0 + 3:-1], op=OP.mult)
    V.tensor_tensor(out=adj9[:, 2:6:3], in0=m1, in1=m2, op=OP.subtract)
    # A12 = h5 h4 - h3 h2 -> row 1 ; A33 = h0 h1 - h3^2 -> row 8
    V.tensor_tensor(out=m1, in0=smx[:, HS0 + 5:HS0 - 1:-5], in1=smx[:, HS0 + 4:HS0:-3],
                    op=OP.mult)
    V.tensor_tensor(out=m2, in0=b2(h(3)), in1=smx[:, HS0 + 2:HS0 + 4], op=OP.mult)
    V.tensor_tensor(out=adj9[:, 1:9:7], in0=m1, in1=m2, op=OP.subtract)
    # det' = h0 A11 + h3 A12 + h5 A13 (fp32 accumulation)
    V.tensor_tensor(out=detf[:, D1R], in0=h(0), in1=adj9[:, 0], op=OP.mult)
    V.tensor_tensor(out=detf[:, D2R], in0=h(3), in1=adj9[:, 1], op=OP.mult)
    V.tensor_tensor(out=detf[:, D3R], in0=h(5), in1=adj9[:, 2], op=OP.mult)
    V.tensor_tensor(out=detf[:, DET_R], in0=detf[:, D1R], in1=detf[:, D2R], op=OP.add)
    V.tensor_tensor(out=detf[:, DET_R], in0=detf[:, DET_R], in1=detf[:, D3R],
                    op=OP.add)
    # mirror rows (ScalarE): A12->3, A13->6, A23->7
    S.copy(out=adj9[:, 3], in_=adj9[:, 1])
    S.copy(out=adj9[:, 6], in_=adj9[:, 2])
    S.copy(out=adj9[:, 7], in_=adj9[:, 5])
    V.reciprocal_approx_fast(out=detf[:, RDN_R], in_=detf[:, DET_R])
    rdn9 = bc(detf[:, RDN_R].unsqueeze(1), (P, 9, J))
    V.scalar_tensor_tensor(out=A9[:].rearrange("p a b j -> p (a b) j"),
                           in0=adj9[:], scalar=-4.0, in1=rdn9,
                           op0=OP.mult, op1=OP.mult)

    # ---------------- bot = A @ Ht' ----------------
    botP4 = botP[:].rearrange("p (r c) a j -> p r c a j", r=3)
    for r in range(3):
        V.tensor_tensor(out=botP4[:, r],
                        in0=bc(A9[:, :, r].unsqueeze(2), (P, 3, 7, J)),
                        in1=bc(Ht[:, r].unsqueeze(1), (P, 3, 7, J)), op=OP.mult)
    V.tensor_tensor(out=botp2[:], in0=botP4[:, 0], in1=botP4[:, 1], op=OP.add)
    V.tensor_tensor(out=bot5[:, 0:3], in0=botp2[:], in1=botP4[:, 2], op=OP.add)
    V.tensor_copy(out=bot5[:, 3:5], in_=bot5[:, 0:2])
    nc.scalar.dma_start(out=dram["outB"][:], in_=bot5[:, 0:3])

    # ---------------- top = jsmS + c x bot ----------------
    cb14 = bc(cb5[:, 1:4].unsqueeze(2), (P, 3, 7, J))
    cb25 = bc(cb5[:, 2:5].unsqueeze(2), (P, 3, 7, J))
    V.tensor_tensor(out=tu[:], in0=cb14, in1=bot5[:, 2:5], op=OP.mult)
    V.tensor_tensor(out=tu2[:], in0=cb25, in1=bot5[:, 1:4], op=OP.mult)
    V.tensor_tensor(out=tu[:], in0=tu[:], in1=tu2[:], op=OP.subtract)
    # final add split on acts so the first outT half DMAs during the second add
    V.tensor_tensor(out=top[:, :, 0:4], in0=jsmS[:, :, 0:4], in1=tu[:, :, 0:4],
                    op=OP.add)
    nc.sync.dma_start(out=dram["outT"][:, :, 0:4], in_=top[:, :, 0:4])
    V.tensor_tensor(out=top[:, :, 4:7], in0=jsmS[:, :, 4:7], in1=tu[:, :, 4:7],
                    op=OP.add)
    nc.scalar.dma_start(out=dram["outT"][:, :, 4:7], in_=top[:, :, 4:7])


@functools.lru_cache(maxsize=1)
def _program():
    from contextlib import ExitStack
    import concourse.bacc as bacc
    import concourse.tile as tile
    from concourse import mybir

    f16 = mybir.dt.float16
    nc = bacc.Bacc("TRN2", target_bir_lowering=False, debug=False)
    dram = {
        "crp": nc.dram_tensor("crp", [P, 9, 7, J], f16, kind="ExternalInput"),
        "pmd": nc.dram_tensor("pmd", [P, 9, 7, J], f16, kind="ExternalInput"),
        "outB": nc.dram_tensor("outB", [P, 3, 7, J], f16, kind="ExternalOutput"),
        "outT": nc.dram_tensor("outT", [P, 3, 7, J], f16, kind="ExternalOutput"),
    }
    with tile.TileContext(nc) as tc:
        with ExitStack() as ctx:
            _emit(nc, tc, ctx, dram)
    nc.compile()
    return nc


@functools.lru_cache(maxsize=1)
def _pmd_consts():
    """pmd const rows 3..8 (P,6,7,J) and the crp cdt row (P,7,J), fp16."""
    rows = np.zeros((6, 7, J), np.float32)
    rows[0] = (MASS * SC)[:, None]
    rows[1] = (SM * SC)[:, None]
    rows[2] = (SM * (-4.0 / TM))[:, None]
    rows[3:6] = (D_SUF.T * SC)[:, :, None]
    cdt_row = np.zeros((7, J), np.float32)
    cdt_row[0:3] = (CD * SC)[:, None]
    pmd_c = np.broadcast_to(rows.astype(np.float16)[None], (P, 6, 7, J))
    cdt_c = np.broadcast_to(cdt_row.astype(np.float16)[None], (P, 7, J))
    return pmd_c, cdt_c


def prepare_in_maps(com_list, pose_list):
    """Host-side marshalling: gather/scale/cast/transpose/pack."""
    com_r = com_list.reshape(N_CORES, P, J, 3, N_ACT)
    com16 = com_r.transpose(0, 1, 3, 4, 2).astype(np.float16)
    pose_r = pose_list.reshape(N_CORES, P, J, 4, 4, 9)
    rotg = pose_r[:, :, :, :3, AXIS, np.arange(N_ACT)]       # (k,P,J,3,7)
    rot16 = (rotg * (SIGN * RS).astype(np.float32)).transpose(0, 1, 3, 4, 2) \
        .astype(np.float16)
    posg = pose_r[:, :, :, :3, 3, :N_ACT]
    pos16 = posg.transpose(0, 1, 3, 4, 2).astype(np.float16)
    pmd_c, cdt_c = _pmd_consts()

    crp = np.empty((N_CORES, P, 9, 7, J), np.float16)
    pmd = np.empty((N_CORES, P, 9, 7, J), np.float16)
    crp[:, :, 0:3] = com16
    crp[:, :, 3:6] = rot16
    crp[:, :, 6:8] = rot16[:, :, 0:2]
    crp[:, :, 8] = cdt_c[None]
    pmd[:, :, 0:3] = pos16
    pmd[:, :, 3:9] = pmd_c[None]
    return [dict(crp=crp[k], pmd=pmd[k]) for k in range(N_CORES)]


def postprocess(results):
    """(k, P, 3, 7, J) fp16 outT/outB -> (512,256,6,7) fp32."""
    out = np.empty((N_CORES, P, J, 6, N_ACT), np.float32)
    for k in range(N_CORES):
        out[k, :, :, 0:3] = results[k]["outT"].astype(np.float32).transpose(0, 3, 1, 2)
        out[k, :, :, 3:6] = results[k]["outB"].astype(np.float32).transpose(0, 3, 1, 2)
    return out.reshape(512, 256, 6, N_ACT)


def _kernel_bm0(com, pose):
    # bm=0 path (not exercised by the shipped setup_inputs; numpy fallback)
    rot = pose[:, :, :3, 2, :N_ACT].copy()
    rot[..., 1] = pose[:, :, :3, 0, 1]
    rot[..., 5] = pose[:, :, :3, 0, 5]
    rot[..., 4] *= -1.0
    delp = pose[:, :, :3, 3, -2][..., None] - pose[:, :, :3, 3, :N_ACT]
    jt = np.cross(rot, delp, axis=2)
    return np.concatenate([jt, rot], axis=2).astype(np.float32)


def kernel(com_list, link_pose_list, bm):
    com_list = np.ascontiguousarray(com_list, dtype=np.float32)
    link_pose_list = np.ascontiguousarray(link_pose_list, dtype=np.float32)
    if not int(bm):
        return _kernel_bm0(com_list, link_pose_list)

    from concourse.bass_utils import run_bass_kernel_spmd

    nc = _program()
    in_maps = prepare_in_maps(com_list, link_pose_list)
    res = run_bass_kernel_spmd(nc, in_maps, core_ids=list(range(N_CORES)))
    return postprocess(res.results)
